# revision 24
# baseline (speedup 1.0000x reference)
"""Trainium2 Bass kernel for nn_ConvchannelAttentionBlock.

reference (per batch b):
    S      = x @ x.T                      (C x C, symmetric; contraction over L)
    probs  = softmax(rowmax(S) - S)       == exp(rowmin(S) - S) / rowsum(...)
    read   = probs @ x                    (C x L)
    out    = eta * read + x

Sharding: data-parallel over B. Each of the 8 cores gets 4 batches and
runs an identical NEFF (SPMD) on its shard; outputs are concatenated.

Per-core pipeline (per batch):
  1. DMA x (f32) -> SBUF; cast to bf16 on GPSIMD.
  2. Build xT (bf16) via PE transposes (128x128 blocks) -> PSUM -> SBUF.
  3. mm1: S = xT.T @ xT accumulated over 32 K-tiles into PSUM (f32).
  4. softmax: rowmin on DVE, E = exp(rowmin - S) on ACT (bf16 out) with
     fused row-sum accumulator Z; s = eta / Z on DVE.
  5. E^T via PE transposes.
  6. mm2: R = E^T.T @ x_bf16 accumulated over 4 K-tiles into PSUM.
  7. out = s * R (ACT scale-copy) + x_f32 (DVE add), DMA out.
All matmul operands are bf16 (1 cycle/row on the PE); accumulation and the
final residual add are f32, so with eta == 0 the output equals x exactly.
"""

import sys

if "/opt/trn_rl_repo" not in sys.path:
    sys.path.insert(0, "/opt/trn_rl_repo")

import numpy as np
import ml_dtypes

import concourse.bacc as bacc
import concourse.tile as tile
from concourse import mybir

B, C, L = 32, 512, 4096
N_CORES = 8
NB = B // N_CORES  # batches per core
P = 128            # partitions
NT = 512           # matmul moving free dim / PSUM bank (f32)

_F32 = mybir.dt.float32
_BF16 = mybir.dt.bfloat16


def build_nc(nb=NB, c=C, l=L):
    """Build the per-core Bass kernel (nb batches of [c, l])."""
    cm = c // P
    ln = l // NT
    lk = l // P

    nc = bacc.Bacc("TRN2", target_bir_lowering=False, debug=False)
    x_d = nc.dram_tensor("x", [nb, c, l], _F32, kind="ExternalInput").ap()
    eta_d = nc.dram_tensor("eta128", [P, 1], _F32, kind="ExternalInput").ap()
    id_d = nc.dram_tensor("ident", [P, P], _BF16, kind="ExternalInput").ap()
    out_d = nc.dram_tensor("out", [nb, c, l], _F32, kind="ExternalOutput").ap()

    with tile.TileContext(nc) as tc:
        with (
            tc.tile_pool(name="const", bufs=1) as const_pool,
            tc.tile_pool(name="xf", bufs=5) as xf_pool,
            tc.tile_pool(name="xb", bufs=6) as xb_pool,
            tc.tile_pool(name="xT", bufs=9) as xT_pool,
            tc.tile_pool(name="ee", bufs=5) as e_pool,
            tc.tile_pool(name="et", bufs=5) as et_pool,
            tc.tile_pool(name="stg", bufs=5) as st_pool,
            tc.tile_pool(name="stat", bufs=4 * cm + 4) as stat_pool,
            tc.tile_pool(name="pT", bufs=2, space="PSUM") as pT_pool,
            tc.tile_pool(name="pS", bufs=2, space="PSUM") as pS_pool,
            tc.tile_pool(name="pE", bufs=2, space="PSUM") as pE_pool,
            tc.tile_pool(name="pR", bufs=2, space="PSUM") as pR_pool,
        ):
            ident = const_pool.tile([P, P], _BF16, tag="ident")
            nc.sync.dma_start(ident[:], id_d[:, :])
            eta = const_pool.tile([P, 1], _F32, tag="eta")
            nc.sync.dma_start(eta[:], eta_d[:, :])

            for b in range(nb):
                # ---- load + cast ----
                xf = []
                xb = []
                for m in range(cm):
                    t = xf_pool.tile([P, l], _F32, tag="xf")
                    nc.sync.dma_start(t[:], x_d[b, m * P:(m + 1) * P, :])
                    xf.append(t)
                    tb = xb_pool.tile([P, l], _BF16, tag="xb")
                    nc.gpsimd.tensor_copy(tb[:], t[:])
                    xb.append(tb)

                # ---- xT via PE transposes ----
                # XT[j] is [P, 4*c]: sub-tile i (free offset i*c) is the
                # [l-block 4j+i (P rows), c] operand tile for mm1.
                XT = [xT_pool.tile([P, 4 * c], _BF16, tag="xT",
                                   name=f"XT_{b}_{j}")
                      for j in range(lk // 4)]
                for m in range(cm):
                    for j in range(lk // 4):
                        pt = pT_pool.tile([P, 4 * P], _BF16, tag="pT")
                        for i in range(4):
                            lb = 4 * j + i
                            nc.tensor.transpose(
                                pt[:, i * P:(i + 1) * P],
                                xb[m][:, lb * P:(lb + 1) * P],
                                ident[:],
                            )
                        # scatter the 4 pieces into XT[j] (one copy, 3D AP)
                        src = pt[:].rearrange("p (i q) -> p i q", i=4)
                        dst = XT[j][:].rearrange("p (i q) -> p i q", i=4)[
                            :, :, m * P:(m + 1) * P]
                        if m % 2 == 0:
                            nc.vector.tensor_copy(dst, src)
                        else:
                            nc.scalar.copy(dst, src)

                def xt_tile(k):
                    return XT[k // 4][:, (k % 4) * c:(k % 4 + 1) * c]

                # ---- mm1 + softmax ----
                E = []
                svec = []
                for m in range(cm):
                    ps = pS_pool.tile([P, c], _F32, tag="pS")
                    for k in range(lk):
                        nc.tensor.matmul(
                            ps[:],
                            xt_tile(k)[:, m * P:(m + 1) * P],
                            xt_tile(k),
                            start=(k == 0),
                            stop=(k == lk - 1),
                        )
                    mn = stat_pool.tile([P, 1], _F32, tag="stat")
                    nc.vector.tensor_reduce(
                        mn[:], ps[:], axis=mybir.AxisListType.X,
                        op=mybir.AluOpType.min)
                    e_t = e_pool.tile([P, c], _BF16, tag="ee")
                    z_t = stat_pool.tile([P, 1], _F32, tag="stat")
                    nc.scalar.activation(
                        e_t[:], ps[:], mybir.ActivationFunctionType.Exp,
                        bias=mn[:], scale=-1.0, accum_out=z_t[:])
                    r_t = stat_pool.tile([P, 1], _F32, tag="stat")
                    nc.vector.reciprocal(r_t[:], z_t[:])
                    s_t = stat_pool.tile([P, 1], _F32, tag="stat")
                    nc.vector.tensor_tensor(
                        s_t[:], eta[:], r_t[:], op=mybir.AluOpType.mult)
                    E.append(e_t)
                    svec.append(s_t)

                # ---- E^T via PE transposes ----
                ET = []
                for dm in range(cm):
                    pe = pE_pool.tile([P, c], _BF16, tag="pE")
                    for cmi in range(cm):
                        nc.tensor.transpose(
                            pe[:, cmi * P:(cmi + 1) * P],
                            E[cmi][:, dm * P:(dm + 1) * P],
                            ident[:],
                        )
                    et_t = et_pool.tile([P, c], _BF16, tag="et")
                    nc.vector.tensor_copy(et_t[:], pe[:])
                    ET.append(et_t)

                # ---- mm2 + epilogue ----
                for m in range(cm):
                    for n in range(ln):
                        pr = pR_pool.tile([P, NT], _F32, tag="pR")
                        for k in range(cm):
                            nc.tensor.matmul(
                                pr[:],
                                ET[k][:, m * P:(m + 1) * P],
                                xb[k][:, n * NT:(n + 1) * NT],
                                start=(k == 0),
                                stop=(k == cm - 1),
                            )
                        stg = st_pool.tile([P, NT], _F32, tag="stg")
                        nc.scalar.mul(stg[:], pr[:], svec[m][:])
                        nc.vector.tensor_tensor(
                            stg[:], stg[:], xf[m][:, n * NT:(n + 1) * NT],
                            op=mybir.AluOpType.add)
                        nc.sync.dma_start(
                            out_d[b, m * P:(m + 1) * P, n * NT:(n + 1) * NT],
                            stg[:])
    nc.compile()
    return nc


_NC_CACHE = {}


def _get_nc():
    if "nc" not in _NC_CACHE:
        _NC_CACHE["nc"] = build_nc()
    return _NC_CACHE["nc"]


def kernel(minibatch: np.ndarray, eta: np.ndarray) -> np.ndarray:
    from concourse.bass_utils import run_bass_kernel_spmd

    assert minibatch.shape == (B, C, L)
    nc = _get_nc()
    eta128 = np.ascontiguousarray(
        np.broadcast_to(eta.reshape(1, 1).astype(np.float32), (P, 1)))
    ident = np.eye(P, dtype=ml_dtypes.bfloat16)
    in_maps = []
    for i in range(N_CORES):
        in_maps.append({
            "x": np.ascontiguousarray(
                minibatch[i * NB:(i + 1) * NB].astype(np.float32)),
            "eta128": eta128,
            "ident": ident,
        })
    res = run_bass_kernel_spmd(nc, in_maps, core_ids=list(range(N_CORES)))
    out = np.concatenate([res.results[i]["out"] for i in range(N_CORES)],
                         axis=0)
    return out.astype(np.float32)
